# revision 1
# baseline (speedup 1.0000x reference)
"""Trainium2 Bass kernel for nn_Attention_1090921693811.

Self-contained: builds an 8-core SPMD Bass graph (one batch sample per
NeuronCore — pure data parallelism), runs it via
concourse.bass_utils.run_bass_kernel_spmd, and gathers the full output.

Per-core pipeline (hardcoded shapes: x[192,128,128], 4 heads x 48 ch):
  conv1x1 GEMM (float32r) -> padded fp16 buffer -> depthwise 3x3 as
  diagonal-stationary TensorE matmuls (PSUM f32 accumulation) ->
  q,k: PE transpose -> Gram accumulation in PSUM; v: kept resident in SBUF.
  Norms + temperature + top-k (max8/match_replace) + masked-softmax combine
  -> mhatT = (Wproj @ blockdiag(A))^T -> out = mhatT.T @ v_dw.
"""
import sys
sys.path.insert(0, '/opt/trn_rl_repo')

import numpy as np
from contextlib import ExitStack
from concourse import bass, bacc, mybir, tile
from concourse.bass_utils import run_bass_kernel_spmd

F32 = mybir.dt.float32
F32R = mybir.dt.float32r
FP16 = mybir.dt.float16
Alu = mybir.AluOpType
Act = mybir.ActivationFunctionType

B = 8; C = 192; C3 = 576; HEADS = 4; CH = 48; H = 128; W = 128; N = H * W
R = 16
NS = H // R
SROWS = R + 2
STRIDE = 130
ABUF = 2 + (SROWS + 1) * STRIDE
TOPKS = (24, 32, 36, 38)
NEG = -1e30
OT = [(0, 128), (128, 128), (256, 128), (384, 128), (512, 64)]

_CACHE = {}


def _host_prep(x, w_qkv, w_dw, w_proj, temperature, attn1, attn2, attn3, attn4):
    x = np.asarray(x, np.float32).reshape(C, N)
    wq = np.asarray(w_qkv, np.float32).reshape(3 * C, C)
    wdw = np.asarray(w_dw, np.float32).reshape(3 * C, 9)
    wp = np.asarray(w_proj, np.float32).reshape(C, C)
    temp = np.asarray(temperature, np.float32).reshape(HEADS)
    wgts = np.stack([np.float32(np.asarray(a).reshape(())) for a in
                     (attn1, attn2, attn3, attn4)])
    d = {"x": x, "wqkvT": np.ascontiguousarray(wq.T)}
    for i, (o0, ow) in enumerate(OT):
        dg = np.zeros((ow, 9 * ow), np.float16)
        for t in range(9):
            dg[np.arange(ow), t * ow + np.arange(ow)] = wdw[o0:o0 + ow, t].astype(np.float16)
        d[f"diag{i}"] = dg
    d["ident"] = np.eye(128, dtype=np.float16)
    wpt = np.zeros((CH, HEADS * C), np.float16)
    for h in range(HEADS):
        wpt[:, h * C:(h + 1) * C] = wp.T[h * CH:(h + 1) * CH, :].astype(np.float16)
    d["wprojT"] = wpt
    d["temp_rep"] = np.ascontiguousarray(np.broadcast_to(temp[None, :], (CH, HEADS))).astype(np.float32)
    d["wgt_rep"] = np.ascontiguousarray(np.broadcast_to(wgts[None, :], (CH, 4))).astype(np.float32)
    d["ones1"] = np.ones((1, CH), np.float32)
    return d


def _build():
    nc = bacc.Bacc("TRN2", target_bir_lowering=False)
    E = {}
    specs = [("x", [C, N], F32R), ("wqkvT", [C, C3], F32R),
             ("ident", [128, 128], FP16), ("wprojT", [CH, HEADS * C], FP16),
             ("temp_rep", [CH, HEADS], F32), ("wgt_rep", [CH, 4], F32),
             ("ones1", [1, CH], F32)]
    for i, (o0, ow) in enumerate(OT):
        specs.append((f"diag{i}", [ow, 9 * ow], FP16))
    for name, shape, dt in specs:
        E[name] = nc.declare_dram_parameter(name, shape, dt, isOutput=False)
    out_ext = nc.declare_dram_parameter("out", [C, N], F32, isOutput=True)

    with tile.TileContext(nc) as tc, ExitStack() as ctx:
        persist = ctx.enter_context(tc.tile_pool(name="persist", bufs=1))
        wqkvT = [persist.tile([128, C3], F32R, tag="wq0", name="wq0"),
                 persist.tile([64, C3], F32R, tag="wq1", name="wq1")]
        nc.sync.dma_start(wqkvT[0][:], E["wqkvT"][0:128, :])
        nc.sync.dma_start(wqkvT[1][:], E["wqkvT"][128:192, :])
        diags = []
        for i, (o0, ow) in enumerate(OT):
            t_ = persist.tile([ow, 9 * ow], FP16, tag=f"dg{i}", name=f"dg{i}")
            nc.sync.dma_start(t_[:], E[f"diag{i}"][:])
            diags.append(t_)
        ident = persist.tile([128, 128], FP16, tag="id", name="id")
        wprojT = persist.tile([CH, HEADS * C], FP16, tag="wpt", name="wpt")
        temp_rep = persist.tile([CH, HEADS], F32, tag="tmp_r", name="tmp_r")
        wgt_rep = persist.tile([CH, 4], F32, tag="wgt_r", name="wgt_r")
        ones1 = persist.tile([1, CH], F32, tag="on1", name="on1")
        for t_, name in ((ident, "ident"), (wprojT, "wprojT"), (temp_rep, "temp_rep"),
                         (wgt_rep, "wgt_rep"), (ones1, "ones1")):
            nc.sync.dma_start(t_[:], E[name][:])
        sumsq = persist.tile([128, 3 * NS], F32, tag="ssq", name="ssq")
        v_dw = [persist.tile([128, N], FP16, tag="vdw0", name="vdw0"),
                persist.tile([128, N // 2], FP16, tag="vdw1", name="vdw1")]
        mid = ctx.enter_context(tc.tile_pool(name="mid", bufs=1))

        def load_x_stripe(s, pool):
            r0 = max(s * R - 1, 0)
            r1 = min(s * R + R + 1, H)
            br0 = r0 - (s * R - 1)
            nr = r1 - r0
            xa = pool.tile([128, SROWS * W], F32R, tag="xa", name="xa")
            xb = pool.tile([64, SROWS * W], F32R, tag="xb", name="xb")
            if br0 > 0:
                nc.gpsimd.memset(xa[:, 0:W].bitcast(F32), 0.0)
                nc.gpsimd.memset(xb[:, 0:W].bitcast(F32), 0.0)
            if br0 + nr < SROWS:
                nc.gpsimd.memset(xa[:, (SROWS - 1) * W:].bitcast(F32), 0.0)
                nc.gpsimd.memset(xb[:, (SROWS - 1) * W:].bitcast(F32), 0.0)
            nc.sync.dma_start(xa[:, br0 * W:(br0 + nr) * W], E["x"][0:128, r0 * W:r1 * W])
            nc.sync.dma_start(xb[:, br0 * W:(br0 + nr) * W], E["x"][128:192, r0 * W:r1 * W])
            return xa, xb

        with tc.tile_pool(name="p1x", bufs=2) as xp, \
             tc.tile_pool(name="p1gps", bufs=2, space="PSUM") as gps, \
             tc.tile_pool(name="p1ab", bufs=3) as abp, \
             tc.tile_pool(name="p1dw", bufs=3) as dwp, \
             tc.tile_pool(name="p1t", bufs=2) as tp, \
             tc.tile_pool(name="peops", bufs=3, space="PSUM") as pps, \
             tc.tile_pool(name="gramp", bufs=1, space="PSUM") as gram_pool:
            gram_ps = gram_pool.tile([CH, HEADS * CH], F32, name="gram_ps")
            nc.vector.memset(gram_ps[:], 0.0)
            for s in range(NS):
                xa, xb = load_x_stripe(s, xp)
                qkT = tp.tile([128, R * 384 + 256], FP16, tag="qkT", name="qkT", bufs=2)
                for i in range(5):
                    o0, ow = OT[i]
                    A = abp.tile([128, ABUF], FP16, tag="A", name="A")
                    nc.gpsimd.memset(A[:, 0:2], 0.0)
                    nc.gpsimd.memset(A[:, 2:2 + SROWS * STRIDE].rearrange(
                        "p (r c) -> p r c", c=STRIDE)[:, :, 128:130], 0.0)
                    ncols = SROWS * W
                    for g0 in range(0, ncols, 1024):
                        gw = min(1024, ncols - g0)
                        pg = gps.tile([128, 1024], F32, tag="g", name="g")
                        for c0 in range(0, gw, 512):
                            cw = min(512, gw - c0)
                            for mi, (mt, xs) in enumerate(((wqkvT[0], xa), (wqkvT[1], xb))):
                                nc.tensor.matmul(pg[:ow, c0:c0 + cw], mt[:, o0:o0 + ow],
                                                 xs[:, g0 + c0:g0 + c0 + cw],
                                                 start=(mi == 0), stop=(mi == 1))
                        rr, nrow = g0 // W, gw // W
                        dstA = A[:ow, 2 + rr * STRIDE:2 + (rr + nrow) * STRIDE].rearrange(
                            "p (r c) -> p r c", c=STRIDE)[:, :, 0:128]
                        nc.scalar.copy(dstA, pg[:ow, 0:nrow * W].rearrange("p (r c) -> p r c", c=W))
                    dgt = diags[i]
                    dense = (dwp.tile([128, R * W], FP16, tag="dw", name="dw")
                             if i < 3 else None)
                    for ch0 in range(0, R, 4):
                        pv = pps.tile([128, 512], F32, tag="pe", name="pe")
                        first = True
                        for dy in (-1, 0, 1):
                            for dx in (-1, 0, 1):
                                t = (dy + 1) * 3 + (dx + 1)
                                base = 2 + (1 + ch0 + dy) * STRIDE + dx
                                mov = A[:ow, base:base + 4 * STRIDE].rearrange(
                                    "p (r c) -> p r c", c=STRIDE)[:, :, 0:128]
                                nc.tensor.matmul(pv[:ow, :].rearrange("p (r c) -> p r c", c=W),
                                                 dgt[:, t * ow:(t + 1) * ow], mov,
                                                 start=first, stop=(t == 8), skip_group_check=True)
                                first = False
                        c0_ = (s * R + ch0) * W
                        if i < 3:
                            nc.vector.tensor_copy(dense[:ow, ch0 * W:(ch0 + 4) * W], pv[:ow, :])
                        elif i == 3:
                            nc.vector.tensor_copy(v_dw[0][:ow, c0_:c0_ + 4 * W], pv[:ow, :])
                        elif c0_ < N // 2:
                            nc.vector.tensor_copy(v_dw[1][0:64, c0_:c0_ + 4 * W], pv[:ow, :])
                        else:
                            nc.vector.tensor_copy(v_dw[1][64:128, c0_ - N // 2:c0_ - N // 2 + 4 * W],
                                                  pv[:ow, :])
                    if i < 3:
                        sq = dwp.tile([128, R * W], FP16, tag="sq", name="sq")
                        nc.scalar.activation(sq[:ow], dense[:ow], Act.Square,
                                             accum_out=sumsq[:ow, i * NS + s:i * NS + s + 1])
                        for rcg in range(0, R, 4):
                            pt = pps.tile([128, 512], FP16, tag="pe", name="pt")
                            for j in range(4):
                                nc.tensor.transpose(pt[:, j * 128:j * 128 + ow],
                                                    dense[:ow, (rcg + j) * 128:(rcg + j + 1) * 128],
                                                    ident[:])
                            off = rcg * 384 + i * 128
                            dst = qkT[:, off:off + 4 * 384].rearrange(
                                "p (r c) -> p r c", c=384)[:, 0:4, 0:ow]
                            nc.vector.tensor_copy(dst, pt[:].rearrange(
                                "p (r c) -> p r c", c=128)[:, :, 0:ow])
                for rc in range(R):
                    for h in range(HEADS):
                        nc.tensor.matmul(gram_ps[:, h * CH:(h + 1) * CH],
                                         qkT[:, rc * 384 + h * CH: rc * 384 + (h + 1) * CH],
                                         qkT[:, rc * 384 + 192 + h * CH: rc * 384 + 192 + (h + 1) * CH],
                                         start=False, stop=(s == NS - 1 and rc == R - 1),
                                         skip_group_check=True)
            gram_sb = mid.tile([CH, HEADS * CH], F32, tag="gramsb", name="gram_sb")
            nc.vector.tensor_copy(gram_sb[:], gram_ps[:])

        # ---- mid: norms + top-k + masked softmax -> mhatT ----
        ssq_col = mid.tile([128, 3], F32, tag="ssqc", name="ssqc")
        for i in range(3):
            nc.vector.tensor_reduce(ssq_col[:, i:i + 1], sumsq[:, i * NS:(i + 1) * NS],
                                    mybir.AxisListType.X, Alu.add)
        rqk = mid.tile([CH, 8], F32, tag="rqk", name="rqk")
        for j in range(8):
            g = j * CH if j < 4 else 192 + (j - 4) * CH
            i, p = divmod(g, 128)
            if p + CH <= 128:
                nc.sync.dma_start(rqk[:, j:j + 1], ssq_col[p:p + CH, i:i + 1])
            else:
                k1 = 128 - p
                nc.sync.dma_start(rqk[0:k1, j:j + 1], ssq_col[p:128, i:i + 1])
                nc.sync.dma_start(rqk[k1:CH, j:j + 1], ssq_col[0:CH - k1, i + 1:i + 2])
        rqk2 = mid.tile([CH, 8], F32, tag="rqk2", name="rqk2")
        nc.scalar.sqrt(rqk2[:], rqk[:])
        nc.vector.reciprocal(rqk[:], rqk2[:])
        rk_row = mid.tile([1, HEADS * CH], F32, tag="rkrow", name="rkrow")
        for h in range(HEADS):
            nc.sync.dma_start(rk_row[0:1, h * CH:(h + 1) * CH], rqk[:, 4 + h:5 + h])
        with tc.tile_pool(name="midps", bufs=1, space="PSUM") as mps:
            rk_rep_ps = mps.tile([CH, HEADS * CH], F32, tag="m", name="rkrep")
            for h in range(HEADS):
                nc.tensor.matmul(rk_rep_ps[:, h * CH:(h + 1) * CH], ones1[:],
                                 rk_row[0:1, h * CH:(h + 1) * CH], start=True, stop=True)
            attn = mid.tile([CH, HEADS * CH], F32, tag="attn", name="attn")
            nc.vector.tensor_tensor(attn[:], gram_sb[:], rk_rep_ps[:], Alu.mult)
            s_col = mid.tile([CH, HEADS], F32, tag="scol", name="scol")
            nc.vector.tensor_tensor(s_col[:], rqk[:, 0:4], temp_rep[:], Alu.mult)
            srt = mid.tile([CH, 5 * 8], F32, tag="srt", name="srt")
            scratch = mid.tile([CH, HEADS * CH], F32, tag="scr", name="scr")
            e_t = mid.tile([CH, HEADS * CH], F32, tag="e", name="e")
            acc_m = mid.tile([CH, HEADS * CH], F32, tag="accm", name="accm")
            mx = mid.tile([CH, 8], F32, tag="mx", name="mx")
            sk = mid.tile([CH, 4], F32, tag="sk", name="sk")
            cf = mid.tile([CH, 4], F32, tag="cf", name="cf")
            junk = mid.tile([CH, CH], F32, tag="junk", name="junk")
            for h in range(HEADS):
                ah = attn[:, h * CH:(h + 1) * CH]
                sc = scratch[:, h * CH:(h + 1) * CH]
                nc.vector.tensor_copy(sc, ah)
                for it in range(5):
                    nc.vector.max(srt[:, it * 8:(it + 1) * 8], sc)
                    if it < 4:
                        nc.vector.match_replace(sc, srt[:, it * 8:(it + 1) * 8], sc, NEG)
                nc.vector.tensor_scalar(mx[:, h:h + 1], srt[:, 0:1], s_col[:, h:h + 1],
                                        -1.0, Alu.mult, Alu.mult)
                eh = e_t[:, h * CH:(h + 1) * CH]
                nc.scalar.activation(eh, ah, Act.Exp, bias=mx[:, h:h + 1], scale=s_col[:, h:h + 1])
                for ki, kk in enumerate(TOPKS):
                    th = srt[:, kk - 1:kk]
                    nc.vector.scalar_tensor_tensor(junk[:], ah, th, eh, Alu.is_ge, Alu.mult,
                                                   accum_out=sk[:, ki:ki + 1])
                nc.vector.reciprocal(sk[:], sk[:])
                nc.vector.tensor_tensor(cf[:], sk[:], wgt_rep[:], Alu.mult)
                am = acc_m[:, h * CH:(h + 1) * CH]
                for ki, kk in enumerate(TOPKS):
                    th = srt[:, kk - 1:kk]
                    if ki == 0:
                        nc.vector.tensor_scalar(am, ah, th, cf[:, ki:ki + 1], Alu.is_ge, Alu.mult)
                    else:
                        nc.vector.tensor_scalar(junk[:], ah, th, cf[:, ki:ki + 1], Alu.is_ge, Alu.mult)
                        nc.vector.tensor_tensor(am, am, junk[:], Alu.add)
                nc.vector.tensor_tensor(am, am, eh, Alu.mult)
            a_bf = mid.tile([CH, HEADS * CH], FP16, tag="abf", name="abf")
            nc.vector.tensor_copy(a_bf[:], acc_m[:])
            mh_sb = mid.tile([CH, HEADS * C], FP16, tag="mhsb", name="mhsb")
            for h in range(HEADS):
                mh_ps = mps.tile([CH, C], F32, tag="m", name="mh_ps")
                nc.tensor.matmul(mh_ps[:], a_bf[:, h * CH:(h + 1) * CH],
                                 wprojT[:, h * C:(h + 1) * C], start=True, stop=True)
                nc.vector.tensor_copy(mh_sb[:, h * C:(h + 1) * C], mh_ps[:])
        mhatT = [mid.tile([128, C], FP16, tag="mhs0", name="mhs0"),
                 mid.tile([128, C], FP16, tag="mhs1", name="mhs1")]
        for h in range(HEADS):
            p0 = h * CH
            if p0 + CH <= 128:
                nc.sync.dma_start(mhatT[0][p0:p0 + CH, :], mh_sb[:, h * C:(h + 1) * C])
            elif p0 >= 128:
                nc.sync.dma_start(mhatT[1][p0 - 128:p0 - 128 + CH, :], mh_sb[:, h * C:(h + 1) * C])
                nc.sync.dma_start(mhatT[1][p0 - 64:p0 - 64 + CH, :], mh_sb[:, h * C:(h + 1) * C])
            else:
                k1 = 128 - p0
                nc.sync.dma_start(mhatT[0][p0:128, :], mh_sb[0:k1, h * C:(h + 1) * C])
                nc.sync.dma_start(mhatT[1][0:CH - k1, :], mh_sb[k1:CH, h * C:(h + 1) * C])
                nc.sync.dma_start(mhatT[1][64:64 + CH - k1, :], mh_sb[k1:CH, h * C:(h + 1) * C])

        with tc.tile_pool(name="p2o", bufs=3) as op, \
             tc.tile_pool(name="p2ops", bufs=4, space="PSUM") as ops_:
            for oo0, oow in ((0, 128), (128, 64)):
                for n0 in range(0, N, 512):
                    po = ops_.tile([128, 512], F32, tag="o", name="o")
                    nc.tensor.matmul(po[:oow, :], mhatT[0][:, oo0:oo0 + oow],
                                     v_dw[0][:, n0:n0 + 512], start=True, stop=False)
                    if n0 < N // 2:
                        nc.tensor.matmul(po[:oow, :], mhatT[1][0:64, oo0:oo0 + oow],
                                         v_dw[1][0:64, n0:n0 + 512], start=False, stop=True)
                    else:
                        nc.tensor.matmul(po[:oow, :], mhatT[1][64:128, oo0:oo0 + oow],
                                         v_dw[1][64:128, n0 - N // 2:n0 - N // 2 + 512],
                                         start=False, stop=True)
                    ot = op.tile([128, 512], F32, tag="ot", name="ot")
                    nc.vector.tensor_copy(ot[:oow, :], po[:oow, :])
                    nc.sync.dma_start(out_ext[oo0:oo0 + oow, n0:n0 + 512], ot[:oow, :])
    nc.finalize()
    return nc


def kernel(**inputs):
    """Full (unsharded) inputs -> full output [8, 192, 128, 128] float32."""
    x = np.asarray(inputs["x"], np.float32)
    if "nc" not in _CACHE:
        _CACHE["nc"] = _build()
    nc = _CACHE["nc"]
    in_maps = [_host_prep(x[b], inputs["w_qkv"], inputs["w_dw"], inputs["w_proj"],
                          inputs["temperature"], inputs["attn1"], inputs["attn2"],
                          inputs["attn3"], inputs["attn4"]) for b in range(B)]
    res = run_bass_kernel_spmd(nc, in_maps, list(range(B)))
    out = np.stack([res.results[b]["out"].reshape(C, H, W) for b in range(B)])
    return out.astype(np.float32)


if __name__ == "__main__":
    rng = np.random.default_rng(0)
    fake = dict(
        x=rng.standard_normal((B, C, H, W), dtype=np.float32),
        w_qkv=rng.standard_normal((3 * C, C, 1, 1), dtype=np.float32) * 0.05,
        w_dw=rng.standard_normal((3 * C, 1, 3, 3), dtype=np.float32) * 0.05,
        w_proj=rng.standard_normal((C, C, 1, 1), dtype=np.float32) * 0.05,
        temperature=np.ones((HEADS, 1, 1), np.float32),
        attn1=np.array([0.2], np.float32), attn2=np.array([0.2], np.float32),
        attn3=np.array([0.2], np.float32), attn4=np.array([0.2], np.float32))
    out = kernel(**fake)
    print("kernel ok:", out.shape, out.dtype)
